# revision 1
# baseline (speedup 1.0000x reference)
"""Trainium2 Bass kernel for CausalHolographicQKV.

Math (validated against reference, rel err ~2e-5):
  All FFT ops move to the frequency domain where they are matmuls (DFT
  bases) + elementwise complex arithmetic.  With E_i = unit-normalized
  half spectra of the five linear projections U_i = x @ (W_i^T F) + fft(b_i):

     out = IDFT( unit(U_a (.) U_b) (.) (unit(U_w)+unit(U_m)+unit(U_rb)) )
           - 3 * cumsum(x, axis=seq)

  The -3*cumsum term (which dominates the output norm) stays in the time
  domain in fp32 (computed by triangular-matrix matmuls in fp32r); the
  small holographic term runs in bf16.  Nyquist bin dropped (error ~3e-5).

Sharding: pure data-parallel over batch (B=8 -> one batch element per core),
cumsum over sequence is fully core-local.  No collectives.
"""

import numpy as np

B, S, D = 8, 2048, 1024
P = 128
NT = S // P          # 16 token tiles per core
ET = D // P          # 8 contraction tiles
NAMES = ["a", "b", "w", "m", "rb"]

_CACHED = {}


def _f32r(ap):
    import dataclasses
    import concourse.mybir as mybir
    return dataclasses.replace(
        ap, tensor=dataclasses.replace(ap.tensor, dtype=mybir.dt.float32r)
    )


def _make_consts():
    import ml_dtypes
    n = np.arange(D)
    k = np.arange(D // 2)
    ang = 2.0 * np.pi * np.outer(n, k) / D          # [D, 512]
    cos = np.cos(ang)
    msin = -np.sin(ang)
    # forward basis: cols 0..511 Re (cos), cols 512..1023 Im (-sin); Im col of
    # bin 0 is all zeros automatically.
    Mbasis = np.concatenate([cos, msin], axis=1).astype(np.float32)  # [D, D]
    # inverse basis: rows 0..511 weight Re, rows 512..1023 weight Im.
    wk = np.where(k == 0, 1.0, 2.0)
    Cinv = (wk[None, :] * np.cos(ang)).T / D         # [512, D]
    Sinv = (-wk[None, :] * np.sin(ang)).T / D        # [512, D]
    Binv = np.concatenate([Cinv, Sinv], axis=0).astype(ml_dtypes.bfloat16)
    return Mbasis, Binv


def _build():
    from contextlib import ExitStack

    import concourse.bass as bass
    import concourse.tile as tile
    from concourse import bacc, mybir

    f32 = mybir.dt.float32
    f32r = mybir.dt.float32r
    bf16 = mybir.dt.bfloat16
    AF = mybir.ActivationFunctionType
    OP = mybir.AluOpType

    import ml_dtypes
    Mbasis_np, Binv_np = _make_consts()
    eye_np = np.eye(P, dtype=np.float32)
    eyeb_np = np.eye(P, dtype=ml_dtypes.bfloat16)
    u = np.arange(P)
    ldiag_np = np.where(u[:, None] <= u[None, :], -3.0, 0.0).astype(np.float32)
    ones_np = np.ones((P, P), dtype=np.float32)
    onesb_np = np.ones((1, P), dtype=ml_dtypes.bfloat16)
    zeros_np = np.zeros((1, D), dtype=np.float32)

    nc = bacc.Bacc("TRN2", target_bir_lowering=False)

    x_d = nc.dram_tensor("x", [S, D], f32r, kind="ExternalInput")
    W_d = {m: nc.dram_tensor(f"W_{m}", [D, D], f32r, kind="ExternalInput")
           for m in NAMES}
    b_d = {m: nc.dram_tensor(f"b_{m}", [D, 1], f32r, kind="ExternalInput")
           for m in NAMES}
    out_d = nc.dram_tensor("out", [S, D], f32, kind="ExternalOutput")
    Mb_d = nc.inline_tensor(Mbasis_np, "Mbasis")
    Bi_d = nc.inline_tensor(Binv_np, "BinvT")
    eye_d = nc.inline_tensor(eye_np, "eye_f32")
    eyeb_d = nc.inline_tensor(eyeb_np, "eye_bf16")
    ldiag_d = nc.inline_tensor(ldiag_np, "ldiag_f32")
    ones_d = nc.inline_tensor(ones_np, "ones_f32")
    onesb_d = nc.inline_tensor(onesb_np, "ones_bf16")
    zeros_d = nc.inline_tensor(zeros_np, "zeros_f32")

    with tile.TileContext(nc) as tc, ExitStack() as ctx:
        const = ctx.enter_context(tc.tile_pool(name="const", bufs=1))
        persist = ctx.enter_context(tc.tile_pool(name="persist", bufs=1))

        ident = const.tile([P, P], f32r)
        nc.sync.dma_start(out=ident[:, :], in_=_f32r(eye_d[:, :]))
        identb = const.tile([P, P], bf16)
        nc.sync.dma_start(out=identb[:, :], in_=eyeb_d[:, :])
        ldiag = const.tile([P, P], f32r)
        nc.sync.dma_start(out=ldiag[:, :], in_=_f32r(ldiag_d[:, :]))
        ones_row = const.tile([1, P], f32r)
        nc.sync.dma_start(out=ones_row[:, :], in_=_f32r(ones_d[0:1, :]))
        ones_row_bf = const.tile([1, P], bf16)
        nc.sync.dma_start(out=ones_row_bf[:, :], in_=onesb_d[:, :])
        ones_col = const.tile([P, 1], f32r)
        nc.sync.dma_start(out=ones_col[:, :], in_=_f32r(ones_d[:, 0:1]))
        bstack = []
        for i, m in enumerate(NAMES):
            bt = const.tile([P, ET, 1], f32r, tag=f"b{i}", name=f"b{i}")
            nc.sync.dma_start(
                out=bt[:, :, :],
                in_=b_d[m][:, :].rearrange("(t p) o -> p t o", p=P),
            )
            bstack.append(bt)

        # persistent combined matrices M_i = W_i^T @ F  (bf16, [d, bins])
        M_sb = [persist.tile([P, ET, D], bf16, tag=f"M{i}", name=f"M{i}") for i in range(5)]
        Bf = [persist.tile([1, D], bf16, tag=f"Bf{i}", name=f"Bf{i}") for i in range(5)]
        P_sb = [persist.tile([1, D], f32r, tag=f"P{i}", name=f"Pst{i}") for i in range(2)]
        nc.sync.dma_start(out=P_sb[0][:, :], in_=_f32r(zeros_d[:, :]))

        # ---------- Phase A: combine  M_i[d, c] = sum_e W_i[e, d] * F[e, c]
        with tc.tile_pool(name="combA", bufs=1) as cpool, \
             tc.tile_pool(name="wA", bufs=10) as wpool, \
             tc.tile_pool(name="psA", bufs=2, space="PSUM") as psA:
            Mb = cpool.tile([P, ET, D], f32r)
            nc.sync.dma_start(
                out=Mb[:, :, :],
                in_=_f32r(Mb_d[:, :].rearrange("(t p) c -> p t c", p=P)))

            for i, m in enumerate(NAMES):
                wt = [wpool.tile([P, D], f32r, tag="w", name="wt") for _ in range(ET)]
                for et in range(ET):
                    nc.sync.dma_start(out=wt[et][:, :],
                                      in_=W_d[m][et * P : (et + 1) * P, :])
                for dg in range(4):          # groups of 2 d-tiles
                    pts = []
                    for dj in range(2):
                        dm = dg * 2 + dj
                        for half in range(2):
                            pt = psA.tile([P, 512], f32, tag=f"ps{dj}_{half}", name=f"ps{dj}_{half}")
                            for et in range(ET):
                                nc.tensor.matmul(
                                    pt[:, :],
                                    wt[et][:, dm * P : (dm + 1) * P],
                                    Mb[:, et, half * 512 : (half + 1) * 512],
                                    start=(et == 0),
                                    stop=(et == ET - 1),
                                )
                            pts.append((dm, half, pt))
                    for idx, (dm, half, pt) in enumerate(pts):
                        eng = nc.vector if idx % 2 == 0 else nc.scalar
                        if eng is nc.vector:
                            eng.tensor_copy(
                                out=M_sb[i][:, dm, half * 512 : (half + 1) * 512],
                                in_=pt[:, :])
                        else:
                            eng.copy(
                                out=M_sb[i][:, dm, half * 512 : (half + 1) * 512],
                                in_=pt[:, :])
                # bias spectrum Bf_i = b_i @ F
                for half in range(2):
                    pb = psA.tile([1, 512], f32, tag="ps0_0", name="pb")
                    for et in range(ET):
                        nc.tensor.matmul(
                            pb[:, :],
                            bstack[i][:, et, 0:1],
                            Mb[:, et, half * 512 : (half + 1) * 512],
                            start=(et == 0),
                            stop=(et == ET - 1),
                        )
                    nc.vector.tensor_copy(
                        out=Bf[i][0:1, half * 512 : (half + 1) * 512],
                        in_=pb[:, :])

        # ---------- Phase B/C: per token tile pipeline
        with tc.tile_pool(name="binv", bufs=1) as bpool, \
             tc.tile_pool(name="xin", bufs=2) as xpool, \
             tc.tile_pool(name="xt", bufs=3) as xtpool, \
             tc.tile_pool(name="ew", bufs=2) as ew, \
             tc.tile_pool(name="crt", bufs=2) as crtpool, \
             tc.tile_pool(name="outp", bufs=2) as outpool, \
             tc.tile_pool(name="psf", bufs=4, space="PSUM") as psf, \
             tc.tile_pool(name="pso", bufs=2, space="PSUM") as pso, \
             tc.tile_pool(name="pss", bufs=1, space="PSUM") as pss:

            Binv = bpool.tile([P, ET, D], bf16)
            nc.sync.dma_start(
                out=Binv[:, :, :],
                in_=Bi_d[:, :].rearrange("(t p) c -> p t c", p=P))


            def act_rsqrt(out, in_):
                eng = nc.scalar
                bias = nc.const_aps.scalar_like(0.0, in_)
                ins = [eng.lower_ap(in_),
                       eng.lower_ap(bias),
                       mybir.ImmediateValue(dtype=mybir.dt.float32, value=1.0),
                       mybir.ImmediateValue(dtype=mybir.dt.float32, value=0.0)]
                return eng.add_instruction(
                    mybir.InstActivation(
                        name=nc.get_next_instruction_name(),
                        func=AF.Rsqrt,
                        ins=ins,
                        outs=[eng.lower_ap(out)],
                    ))

            def vtt(op, a, b, tag, eng=None):
                o = ew.tile([P, 512], bf16, tag=tag, name=tag)
                (eng or nc.vector).tensor_tensor(out=o[:, :], in0=a[:, :],
                                                 in1=b[:, :], op=op)
                return o

            for tt in range(NT):
                xt = xpool.tile([P, D], f32r, tag="x")
                nc.sync.dma_start(out=xt[:, :],
                                  in_=x_d[tt * P : (tt + 1) * P, :])

                # transpose x tile -> xT (bf16) via DMA xbar transpose
                xbf = xtpool.tile([P, D], bf16, tag="xbf", name="xbf")
                nc.vector.tensor_copy(out=xbf[:, :], in_=xt[:, :])
                xTt = xtpool.tile([P, ET, P], bf16, tag="xT")
                for dt in range(ET):
                    nc.sync.dma_start_transpose(
                        out=xTt[:, dt, :],
                        in_=xbf[:, dt * P : (dt + 1) * P])

                # block sum for the running cumsum prefix
                pcur = P_sb[tt % 2]
                pnext = P_sb[(tt + 1) % 2]
                for half in range(2):
                    pb = pss.tile([1, 512], f32, tag="bs")
                    nc.tensor.matmul(
                        pb[:, :],
                        ones_col[:, :],
                        xt[:, half * 512 : (half + 1) * 512],
                        start=True, stop=True,
                    )
                    nc.vector.scalar_tensor_tensor(
                        out=pnext[0:1, half * 512 : (half + 1) * 512],
                        in0=pb[:, :],
                        scalar=-3.0,
                        in1=pcur[0:1, half * 512 : (half + 1) * 512],
                        op0=OP.mult,
                        op1=OP.add,
                    )

                # forward transforms: U_i = xT.T @ M_i + Bf_i
                uplanes = []
                for i in range(5):
                    planes = []
                    for half in range(2):
                        pf = psf.tile([P, 512], f32, tag="fw", name="fw")
                        for dt in range(ET):
                            nc.tensor.matmul(
                                pf[:, :],
                                xTt[:, dt, :],
                                M_sb[i][:, dt, half * 512 : (half + 1) * 512],
                                start=(dt == 0), stop=False,
                            )
                        nc.tensor.matmul(
                            pf[:, :],
                            ones_row_bf[:, :],
                            Bf[i][0:1, half * 512 : (half + 1) * 512],
                            start=False, stop=True,
                        )
                        planes.append(pf)
                    uplanes.append(planes)

                # evacuate to bf16 (a, b raw; w, m, rb normalized)
                sb = {}
                for i in range(5):
                    for half, sfx in ((0, "r"), (1, "i")):
                        tag = (f"u{i}{sfx}" if i < 2 else f"u{sfx}")
                        t = ew.tile([P, 512], bf16, tag=tag, name=tag)
                        if (i + half) % 2 == 0:
                            nc.vector.tensor_copy(out=t[:, :],
                                                  in_=uplanes[i][half][:, :])
                        else:
                            nc.scalar.copy(out=t[:, :],
                                           in_=uplanes[i][half][:, :])
                        sb[(i, sfx)] = t

                er = {}
                ei = {}
                for i in (2, 3, 4):
                    rr, ri = sb[(i, "r")], sb[(i, "i")]
                    q1 = ew.tile([P, 512], bf16, tag="q1", name="q1")
                    nc.scalar.square(q1[:, :], rr[:, :])
                    q2 = ew.tile([P, 512], bf16, tag="q2", name="q2")
                    nc.scalar.square(q2[:, :], ri[:, :])
                    m2 = vtt(OP.add, q1, q2, "m2")
                    inv = ew.tile([P, 512], bf16, tag="inv", name="inv")
                    act_rsqrt(inv[:, :], m2[:, :])
                    er[i] = vtt(OP.mult, rr, inv, f"er{i}")
                    ei[i] = vtt(OP.mult, ri, inv, f"ei{i}")

                ar, ai = sb[(0, "r")], sb[(0, "i")]
                br, bi = sb[(1, "r")], sb[(1, "i")]
                z1 = vtt(OP.mult, ar, br, "q1")
                z2 = vtt(OP.mult, ai, bi, "q2")
                zr = vtt(OP.subtract, z1, z2, "zr")
                z3 = vtt(OP.mult, ar, bi, "q1")
                z4 = vtt(OP.mult, ai, br, "q2")
                zi = vtt(OP.add, z3, z4, "zi")
                q1 = ew.tile([P, 512], bf16, tag="q1", name="q1z")
                nc.scalar.square(q1[:, :], zr[:, :])
                q2 = ew.tile([P, 512], bf16, tag="q2", name="q2z")
                nc.scalar.square(q2[:, :], zi[:, :])
                mz = vtt(OP.add, q1, q2, "m2")
                izv = ew.tile([P, 512], bf16, tag="izv", name="izv")
                act_rsqrt(izv[:, :], mz[:, :])

                s1 = vtt(OP.add, er[2], er[3], "q1")
                srr = vtt(OP.add, s1, er[4], "srr")
                s2 = vtt(OP.add, ei[2], ei[3], "q2")
                sri = vtt(OP.add, s2, ei[4], "sri")

                p1 = vtt(OP.mult, zr, srr, "q1")
                p2 = vtt(OP.mult, zi, sri, "q2")
                pr = vtt(OP.subtract, p1, p2, "pr")
                p3 = vtt(OP.mult, zr, sri, "q1")
                p4 = vtt(OP.mult, zi, srr, "q2")
                pi = vtt(OP.add, p3, p4, "pi")
                crr = vtt(OP.mult, pr, izv, "crr")
                cri = vtt(OP.mult, pi, izv, "cri")

                # transpose CR (tok-major -> bin-major) via DMA xbar transpose
                crt = crtpool.tile([P, ET, P], bf16, tag="crt")
                for jg in range(2):
                    csrc = crr if jg == 0 else cri
                    for jj in range(4):
                        nc.sync.dma_start_transpose(
                            out=crt[:, jg * 4 + jj, :],
                            in_=csrc[:, jj * P : (jj + 1) * P])

                # IDFT + fold in -3*cumsum (triangular + prefix matmuls)
                osb = outpool.tile([P, D], f32, tag="out")
                for half in range(2):
                    po = pso.tile([P, 512], f32, tag="od")
                    for j in range(ET):
                        nc.tensor.matmul(
                            po[:, :],
                            crt[:, j, :],
                            Binv[:, j, half * 512 : (half + 1) * 512],
                            start=(j == 0), stop=False,
                        )
                    nc.tensor.matmul(
                        po[:, :],
                        ldiag[:, :],
                        xt[:, half * 512 : (half + 1) * 512],
                        start=False, stop=False,
                    )
                    nc.tensor.matmul(
                        po[:, :],
                        ones_row[:, :],
                        pcur[0:1, half * 512 : (half + 1) * 512],
                        start=False, stop=True,
                    )
                    nc.scalar.copy(out=osb[:, half * 512 : (half + 1) * 512],
                                   in_=po[:, :])
                nc.sync.dma_start(out=out_d[tt * P : (tt + 1) * P, :],
                                  in_=osb[:, :])

    nc.compile()
    return nc


def _get_nc():
    if "nc" not in _CACHED:
        _CACHED["nc"] = _build()
    return _CACHED["nc"]


def kernel(**inputs):
    from concourse.bass_utils import run_bass_kernel_spmd

    nc = _get_nc()
    x = np.ascontiguousarray(inputs["x"], dtype=np.float32)
    in_maps = []
    for c in range(B):
        m = {"x": x[c]}
        for nm in NAMES:
            m[f"W_{nm}"] = np.ascontiguousarray(inputs[f"W_{nm}"],
                                                dtype=np.float32)
            m[f"b_{nm}"] = np.ascontiguousarray(
                inputs[f"b_{nm}"], dtype=np.float32).reshape(D, 1)
        in_maps.append(m)
    res = run_bass_kernel_spmd(nc, in_maps, core_ids=list(range(B)))
    out = np.stack([r["out"] for r in res.results], axis=0)
    return out.astype(np.float32)

